# revision 5
# baseline (speedup 1.0000x reference)
"""Max-sum belief-propagation kernel for Trainium2 (8 NeuronCores, SPMD).

Problem: B=512 batch of factor graphs, N=64 nodes, E=512 edges, A=16 actions,
8 message-passing iterations.  Data-parallel over batch: 64 batch elems/core.

Device computes the q-value trajectory (scaled by E=512, an exact power of
two, so raw edge_vals bytes are streamed unscaled); the host computes the
argmax/eval/a_max bookkeeping track, which never feeds back into the
dynamics.
"""
import sys

sys.path.insert(0, "/opt/trn_rl_repo")

import numpy as np

MSG_ITERS = 8
B, N, E, A = 512, 64, 512, 16
NCORES = 8
BL = B // NCORES          # 64 batch elements per core
NCHUNK = 4                # e-chunks of 128 edges
EC = E // NCHUNK          # 128
NBH = 2                   # b halves per chunk DMA
BH = BL // NBH            # 32

_prog_cache = {}


def _build_program():
    import concourse.mybir as mybir
    import concourse.tile as tile
    from concourse import bacc

    f32 = mybir.dt.float32
    add = mybir.AluOpType.add
    sub = mybir.AluOpType.subtract

    nc = bacc.Bacc("TRN2", target_bir_lowering=False, debug=False,
                   num_devices=NCORES)

    edge_in = nc.dram_tensor("edge", [E, BL, A, A], f32, kind="ExternalInput")
    q0_in = nc.dram_tensor("q0", [N, BL * A], f32, kind="ExternalInput")
    gf_in = nc.dram_tensor("gf", [N, E], f32, kind="ExternalInput")
    gt_in = nc.dram_tensor("gt", [N, E], f32, kind="ExternalInput")
    sf_in = nc.dram_tensor("sf", [E, N], f32, kind="ExternalInput")
    st_in = nc.dram_tensor("st", [E, N], f32, kind="ExternalInput")
    q_out = nc.dram_tensor("qout", [MSG_ITERS, N, BL * A], f32,
                           kind="ExternalOutput")

    with tile.TileContext(nc) as tc:
        with tc.tile_pool(name="const", bufs=1) as cpool, \
             tc.tile_pool(name="msg", bufs=1) as mpool, \
             tc.tile_pool(name="edge", bufs=2) as epool, \
             tc.tile_pool(name="tmp", bufs=1) as tpool, \
             tc.tile_pool(name="cfcb", bufs=2) as cfpool, \
             tc.tile_pool(name="raw", bufs=2) as rpool, \
             tc.tile_pool(name="small", bufs=4) as spool, \
             tc.tile_pool(name="psg", bufs=2, space="PSUM") as psg, \
             tc.tile_pool(name="psq", bufs=2, space="PSUM") as psq:

            q0_t = cpool.tile([N, BL * A], f32, tag="q0")
            gf_t = cpool.tile([N, E], f32, tag="gf")
            gt_t = cpool.tile([N, E], f32, tag="gt")
            nc.sync.dma_start(q0_t[:], q0_in[:])
            nc.sync.dma_start(gf_t[:], gf_in[:])
            nc.sync.dma_start(gt_t[:], gt_in[:])

            sf_c = []
            st_c = []
            for c in range(NCHUNK):
                s1 = cpool.tile([EC, N], f32, tag=f"sf{c}")
                s2 = cpool.tile([EC, N], f32, tag=f"st{c}")
                nc.sync.dma_start(s1[:], sf_in[c * EC:(c + 1) * EC, :])
                nc.sync.dma_start(s2[:], st_in[c * EC:(c + 1) * EC, :])
                sf_c.append(s1)
                st_c.append(s2)

            q_t = cpool.tile([N, BL * A], f32, tag="q")
            mf_t = [mpool.tile([EC, BL, A], f32, tag=f"mf{c}",
                               name=f"mf{c}") for c in range(NCHUNK)]
            mb_t = [mpool.tile([EC, BL, A], f32, tag=f"mb{c}",
                               name=f"mb{c}") for c in range(NCHUNK)]
            for c in range(NCHUNK):
                nc.vector.memset(mf_t[c][:], 0.0)
                nc.vector.memset(mb_t[c][:], 0.0)

            for k in range(MSG_ITERS):
                qsrc = q0_t if k == 0 else q_t
                qacc = psq.tile([N, BL * A], f32, tag="qacc")
                n_sc = 0  # scatter matmul counter per psum half
                for c in range(NCHUNK):
                    # cf = q[from_e] - mb_prev ; cb = q[to_e] - mf_prev
                    gps = psg.tile([EC, BL * A], f32, tag="gath")
                    for h2 in range(2):
                        nc.tensor.matmul(
                            gps[:, h2 * 512:(h2 + 1) * 512],
                            gf_t[:, c * EC:(c + 1) * EC],
                            qsrc[:, h2 * 512:(h2 + 1) * 512],
                            start=True, stop=True)
                    cf_t = cfpool.tile([EC, BL, A], f32, tag="cf")
                    nc.vector.tensor_tensor(
                        cf_t[:].rearrange("p b a -> p (b a)"), gps[:],
                        mb_t[c][:].rearrange("p b a -> p (b a)"), op=sub)

                    gps2 = psg.tile([EC, BL * A], f32, tag="gath")
                    for h2 in range(2):
                        nc.tensor.matmul(
                            gps2[:, h2 * 512:(h2 + 1) * 512],
                            gt_t[:, c * EC:(c + 1) * EC],
                            qsrc[:, h2 * 512:(h2 + 1) * 512],
                            start=True, stop=True)
                    cb_t = cfpool.tile([EC, BL, A], f32, tag="cb")
                    nc.vector.tensor_tensor(
                        cb_t[:].rearrange("p b a -> p (b a)"), gps2[:],
                        mf_t[c][:].rearrange("p b a -> p (b a)"), op=sub)

                    mfraw = rpool.tile([EC, BL, A], f32, tag="mfraw")
                    mbraw = rpool.tile([EC, BL, A], f32, tag="mbraw")
                    for h in range(NBH):
                        et = epool.tile([EC, BH, A, A], f32, tag="et")
                        nc.sync.dma_start(
                            et[:],
                            edge_in[c * EC:(c + 1) * EC,
                                    h * BH:(h + 1) * BH, :, :])
                        bsl = slice(h * BH, (h + 1) * BH)
                        # forward: tmp = edge + cf[af] (bcast over at);
                        # reduce over af
                        tmp = tpool.tile([EC, BH, A, A], f32, tag="tmp")
                        nc.vector.tensor_tensor(
                            tmp[:], et[:],
                            cf_t[:, bsl, :, None].to_broadcast(
                                (EC, BH, A, A)), op=add)
                        nc.vector.reduce_max(
                            mfraw[:, bsl, :],
                            tmp[:].rearrange("p b af at -> p b at af"),
                            axis=mybir.AxisListType.X)
                        # backward: tmp2 = edge + cb[at] (bcast over af);
                        # reduce over at
                        tmp2 = tpool.tile([EC, BH, A, A], f32, tag="tmp")
                        nc.vector.tensor_tensor(
                            tmp2[:], et[:],
                            cb_t[:, bsl, None, :].to_broadcast(
                                (EC, BH, A, A)), op=add)
                        nc.vector.reduce_max(
                            mbraw[:, bsl, :], tmp2[:],
                            axis=mybir.AxisListType.X)

                    # mean-center and store as the new messages
                    for raw, msg in ((mfraw, mf_t[c]), (mbraw, mb_t[c])):
                        msum = spool.tile([EC, BL], f32, tag="msum")
                        nc.vector.reduce_sum(msum[:], raw[:],
                                             axis=mybir.AxisListType.X)
                        mmean = spool.tile([EC, BL], f32, tag="mmean")
                        nc.vector.tensor_scalar_mul(mmean[:], msum[:],
                                                    1.0 / A)
                        nc.vector.tensor_tensor(
                            msg[:], raw[:],
                            mmean[:, :, None].to_broadcast((EC, BL, A)),
                            op=sub)

                    # scatter: q += S_to.T @ mf  +  S_from.T @ mb
                    for lhs, msg in ((st_c[c], mf_t[c]), (sf_c[c], mb_t[c])):
                        flat = msg[:].rearrange("p b a -> p (b a)")
                        for h2 in range(2):
                            nc.tensor.matmul(
                                qacc[:, h2 * 512:(h2 + 1) * 512],
                                lhs[:],
                                flat[:, h2 * 512:(h2 + 1) * 512],
                                start=(n_sc == 0),
                                stop=(n_sc == 2 * NCHUNK - 1))
                        n_sc += 1

                nc.vector.tensor_tensor(q_t[:], q0_t[:], qacc[:], op=add)
                nc.sync.dma_start(q_out[k, :, :], q_t[:])

    nc.compile()
    return nc


def _get_program():
    if "nc" not in _prog_cache:
        _prog_cache["nc"] = _build_program()
    return _prog_cache["nc"]


def _make_in_maps(node_vals, edge_vals, ef, et):
    gf = np.zeros((N, E), np.float32)
    gf[ef, np.arange(E)] = 1.0
    gt = np.zeros((N, E), np.float32)
    gt[et, np.arange(E)] = 1.0
    sf = np.ascontiguousarray(gf.T)
    st = np.ascontiguousarray(gt.T)
    in_maps = []
    for c in range(NCORES):
        bs = slice(c * BL, (c + 1) * BL)
        edge_r = np.ascontiguousarray(edge_vals[bs].transpose(1, 0, 2, 3))
        q0 = np.ascontiguousarray(
            (node_vals[bs] * np.float32(E / N)).transpose(1, 0, 2)
        ).reshape(N, BL * A)
        in_maps.append({"edge": edge_r, "q0": q0, "gf": gf, "gt": gt,
                        "sf": sf, "st": st})
    return in_maps


def _eval_actions(a, node_vals, edge_vals, ef, et):
    """Vectorized fp32 mirror of reference._eval_action. a: [B, N] int."""
    node_val = np.take_along_axis(
        node_vals, a[:, :, None], axis=-1)[..., 0].sum(-1, dtype=np.float32)
    a_f = a[:, ef]
    a_t = a[:, et]
    b_idx = np.arange(B)[:, None]
    e_idx = np.arange(E)[None, :]
    edge_val = edge_vals[b_idx, e_idx, a_f, a_t].sum(-1, dtype=np.float32)
    return (node_val / np.float32(N) + edge_val / np.float32(E)).astype(
        np.float32)


def _host_track(q_traj, node_vals, edge_vals, ef, et):
    """q_traj: [MSG_ITERS, B, N, A] scaled q values. Returns q_max, a_max."""
    a_max = np.argmax(node_vals, axis=-1).astype(np.int32)
    q_max = _eval_actions(a_max, node_vals, edge_vals, ef, et)
    for k in range(MSG_ITERS):
        a_k = np.argmax(q_traj[k], axis=-1).astype(np.int32)
        q_val = _eval_actions(a_k, node_vals, edge_vals, ef, et)
        upd = q_val > q_max
        a_max = np.where(upd[:, None], a_k, a_max)
        q_max = np.where(upd, q_val, q_max)
    return q_max.astype(np.float32), a_max.astype(np.int32)


def _run_device(in_maps, trace=False):
    from concourse.bass_utils import run_bass_kernel_spmd
    nc = _get_program()
    return run_bass_kernel_spmd(nc, in_maps, core_ids=list(range(NCORES)),
                                trace=trace)


def measure_hw_time(in_maps, n_iters=12):
    """Time repeated device executions (PJRT dispatch included), seconds."""
    import time
    import jax
    import numpy as _np
    from jax.sharding import Mesh, PartitionSpec
    from jax.experimental.shard_map import shard_map
    from concourse import bass2jax
    from concourse import mybir

    nc = _get_program()
    bass2jax.install_neuronx_cc_hook()
    partition_name = (nc.partition_id_tensor.name
                      if nc.partition_id_tensor else None)
    in_names, out_names, out_avals, zero_outs = [], [], [], []
    for alloc in nc.m.functions[0].allocations:
        if not isinstance(alloc, mybir.MemoryLocationSet):
            continue
        name = alloc.memorylocations[0].name
        if alloc.kind == "ExternalInput":
            if name != partition_name:
                in_names.append(name)
        elif alloc.kind == "ExternalOutput":
            shape = tuple(alloc.tensor_shape)
            dtype = mybir.dt.np(alloc.dtype)
            out_names.append(name)
            out_avals.append(jax.core.ShapedArray(shape, dtype))
            zero_outs.append(_np.zeros(shape, dtype))
    n_params = len(in_names)
    n_outs = len(out_avals)
    all_in_names = list(in_names) + list(out_names)
    if partition_name is not None:
        all_in_names.append(partition_name)

    def _body(*args):
        operands = list(args)
        if partition_name is not None:
            operands.append(bass2jax.partition_id_tensor())
        outs = bass2jax._bass_exec_p.bind(
            *operands,
            out_avals=tuple(out_avals),
            in_names=tuple(all_in_names),
            out_names=tuple(out_names),
            lowering_input_output_aliases=(),
            sim_require_finite=True,
            sim_require_nnan=True,
            nc=nc,
        )
        return tuple(outs)

    devices = jax.devices()[:NCORES]
    mesh = Mesh(_np.asarray(devices), ("core",))
    in_specs = (PartitionSpec("core"),) * (n_params + n_outs)
    out_specs = (PartitionSpec("core"),) * n_outs
    donate = tuple(range(n_params, n_params + n_outs))
    sharded = jax.jit(
        shard_map(_body, mesh=mesh, in_specs=in_specs, out_specs=out_specs,
                  check_rep=False),
        donate_argnums=donate, keep_unused=True)

    concat_in = [
        _np.concatenate([_np.asarray(m[name]) for m in in_maps], axis=0)
        for name in in_names
    ]
    sharding = jax.sharding.NamedSharding(mesh, PartitionSpec("core"))
    in_dev = [jax.device_put(a, sharding) for a in concat_in]

    times = []
    out = None
    for i in range(n_iters):
        zs = [jax.device_put(
            _np.zeros((NCORES * z.shape[0], *z.shape[1:]), z.dtype), sharding)
            for z in zero_outs]
        t0 = time.perf_counter()
        out = sharded(*in_dev, *zs)
        jax.block_until_ready(out)
        t1 = time.perf_counter()
        times.append(t1 - t0)
    return times


def kernel(node_vals, edge_vals, edges_from, edges_to):
    node_vals = np.asarray(node_vals, dtype=np.float32)
    edge_vals = np.asarray(edge_vals, dtype=np.float32)
    ef = np.asarray(edges_from, dtype=np.int64)
    et = np.asarray(edges_to, dtype=np.int64)

    in_maps = _make_in_maps(node_vals, edge_vals, ef, et)
    res = _run_device(in_maps)

    q_traj = np.empty((MSG_ITERS, B, N, A), np.float32)
    for c in range(NCORES):
        qo = res.results[c]["qout"].reshape(MSG_ITERS, N, BL, A)
        q_traj[:, c * BL:(c + 1) * BL] = qo.transpose(0, 2, 1, 3)

    q_max, a_max = _host_track(q_traj, node_vals, edge_vals, ef, et)
    return q_max, a_max


# revision 19
# speedup vs baseline: 90.3023x; 90.3023x over previous
"""Max-sum belief-propagation kernel for Trainium2 (8 NeuronCores, SPMD).

Problem: B=512 batch of factor graphs, N=64 nodes, E=512 edges, A=16 actions,
8 message-passing iterations.  Data-parallel over batch: 64 batch elems/core.

Device computes the q-value trajectory (scaled by E=512, an exact power of
two, so raw edge_vals bytes are streamed unscaled); the host computes the
argmax/eval/a_max bookkeeping track, which never feeds back into the
dynamics.
"""
import sys

sys.path.insert(0, "/opt/trn_rl_repo")

import numpy as np

MSG_ITERS = 8
B, N, E, A = 512, 64, 512, 16
NCORES = 8
BL = B // NCORES          # 64 batch elements per core
NCHUNK = 4                # e-chunks of 128 edges
EC = E // NCHUNK          # 128
NBH = 4                   # b slices per chunk DMA
BH = BL // NBH            # 16

_prog_cache = {}

# (chunk, half, dir) -> True to run that add+reduce unit on GPSIMD.
# dir 0 = forward, 1 = backward.  Balanced so DVE and GPSIMD finish together.
# GPSIMD cannot run 2-stream ops under this toolchain; keep empty.
GP_UNITS = {}

# Forward-direction adds run on the (otherwise idle) PE as identity-matmul
# PSUM accumulations: psum = I@edge; psum += I@cf_bcast, which is exactly
# fl(edge + cf) per element (fp32 PSUM adder).  DVE then reduces straight
# from PSUM.  This offloads ~25% of DVE's elementwise work.
PE_FWD_ADD = True
PPC = 4   # psum pieces per (chunk, slice) unit; piece = [EC, BH//PPC * 256]


def _tree_max(eng, tpool, f32, tmp, out_ap, axis):
    """Grouped max over `axis` (2=af, 3=at) of tmp [EC,BH,A,A] via a
    4-level pairwise tree of tensor_tensor(max) ops (exact: max is
    order-invariant).  All operands are rank<=3 APs for Pool-engine
    compatibility."""
    import concourse.mybir as mybir
    mx = mybir.AluOpType.max

    if axis == 2:
        # reduce over af keeping at: flat view [p, b, af*at]; halve the run.
        src = tmp[:].rearrange("p b af at -> p b (af at)")
        n = A * A
        while n > A:
            half = n // 2
            lo = src[:, :, 0:half]
            hi = src[:, :, half:n]
            if half == A:
                dst = out_ap
            else:
                t = tpool.tile([EC, BH, half], f32, tag=f"treef{half}",
                               name=f"treef{half}")
                dst = t[:]
            eng.tensor_tensor(dst, lo, hi, op=mx)
            src = dst
            n = half
    else:
        # reduce over at keeping af: view [p, (b af), at]; halve innermost.
        src = tmp[:].rearrange("p b af at -> p (b af) at")
        out = out_ap.rearrange("p b af -> p (b af)")
        n = A
        while n > 1:
            half = n // 2
            lo = src[:, :, 0:half]
            hi = src[:, :, half:n]
            if half == 1:
                dst = out
                lo = lo[:, :, 0]
                hi = hi[:, :, 0]
            else:
                t = tpool.tile([EC, BH * A, half], f32, tag=f"treeb{half}",
                               name=f"treeb{half}")
                dst = t[:]
            eng.tensor_tensor(dst, lo, hi, op=mx)
            if half > 1:
                src = dst
            n = half


def _build_program(repeat=1):
    import contextlib
    import concourse.mybir as mybir
    import concourse.tile as tile
    from concourse import bacc

    f32 = mybir.dt.float32
    add = mybir.AluOpType.add
    sub = mybir.AluOpType.subtract

    nc = bacc.Bacc("TRN2", target_bir_lowering=False, debug=False,
                   num_devices=NCORES)

    edge_in = nc.dram_tensor("edge", [E, BL, A, A], f32, kind="ExternalInput")
    q0_in = nc.dram_tensor("q0", [N, BL * A], f32, kind="ExternalInput")
    gf_in = nc.dram_tensor("gf", [N, E], f32, kind="ExternalInput")
    gt_in = nc.dram_tensor("gt", [N, E], f32, kind="ExternalInput")
    sf_in = nc.dram_tensor("sf", [E, N], f32, kind="ExternalInput")
    st_in = nc.dram_tensor("st", [E, N], f32, kind="ExternalInput")
    id_in = nc.dram_tensor("ident", [EC, EC], f32, kind="ExternalInput")
    q_out = nc.dram_tensor("qout", [MSG_ITERS, N, BL * A], f32,
                           kind="ExternalOutput")

    with tile.TileContext(nc) as tc:
        with tc.tile_pool(name="const", bufs=1) as cpool, \
             tc.tile_pool(name="msg", bufs=1) as mpool, \
             tc.tile_pool(name="edge", bufs=2) as epool, \
             tc.tile_pool(name="tmp", bufs=3) as tpool, \
             tc.tile_pool(name="tree", bufs=1) as trpool, \
             tc.tile_pool(name="cfcb", bufs=1) as cfpool, \
             tc.tile_pool(name="raw", bufs=2) as rpool, \
             tc.tile_pool(name="small", bufs=4) as spool, \
             tc.tile_pool(name="psg", bufs=1, space="PSUM") as psg, \
             tc.tile_pool(name="psq", bufs=1, space="PSUM") as psq, \
             tc.tile_pool(name="pst", bufs=2, space="PSUM") as pstmp:

            q0_t = cpool.tile([N, BL * A], f32, tag="q0")
            gf_t = cpool.tile([N, E], f32, tag="gf")
            gt_t = cpool.tile([N, E], f32, tag="gt")
            nc.sync.dma_start(q0_t[:], q0_in[:])
            nc.sync.dma_start(gf_t[:], gf_in[:])
            nc.sync.dma_start(gt_t[:], gt_in[:])

            sf_c = []
            st_c = []
            for c in range(NCHUNK):
                s1 = cpool.tile([EC, N], f32, tag=f"sf{c}")
                s2 = cpool.tile([EC, N], f32, tag=f"st{c}")
                nc.sync.dma_start(s1[:], sf_in[c * EC:(c + 1) * EC, :])
                nc.sync.dma_start(s2[:], st_in[c * EC:(c + 1) * EC, :])
                sf_c.append(s1)
                st_c.append(s2)

            ident_t = cpool.tile([EC, EC], f32, tag="ident")
            nc.sync.dma_start(ident_t[:], id_in[:])
            q_t = cpool.tile([N, BL * A], f32, tag="q")
            mf_t = [mpool.tile([EC, BL, A], f32, tag=f"mf{c}",
                               name=f"mf{c}") for c in range(NCHUNK)]
            mb_t = [mpool.tile([EC, BL, A], f32, tag=f"mb{c}",
                               name=f"mb{c}") for c in range(NCHUNK)]

            rep_ctx = (tc.For_i(0, repeat, 1) if repeat > 1
                       else contextlib.nullcontext())
            with rep_ctx:
                for c in range(NCHUNK):
                    nc.vector.memset(mf_t[c][:], 0.0)
                    nc.vector.memset(mb_t[c][:], 0.0)
                _emit_iters(nc, tc, mybir, f32, add, sub, q0_t, q_t, gf_t,
                            gt_t, sf_c, st_c, mf_t, mb_t, edge_in, q_out,
                            epool, tpool, trpool, cfpool, rpool, spool, psg,
                            psq, pstmp, ident_t)

    nc.compile()
    return nc


def _emit_iters(nc, tc, mybir, f32, add, sub, q0_t, q_t, gf_t, gt_t, sf_c,
                st_c, mf_t, mb_t, edge_in, q_out, epool, tpool, trpool,
                cfpool, rpool, spool, psg, psq, pstmp, ident_t):
    if True:
        if True:
            for k in range(MSG_ITERS):
                qsrc = q0_t if k == 0 else q_t
                qacc = psq.tile([N, BL * A], f32, tag="qacc")
                n_sc = 0  # scatter matmul counter per psum half

                # Phase A: all gathers (PE) + cf/cb subs, so the in-order PE
                # never stalls a later chunk's gather behind an earlier
                # chunk's scatter.
                cf_c, cb_c = [], []
                for c in range(NCHUNK):
                    # cf = q[from_e] - mb_prev ; cb = q[to_e] - mf_prev
                    gps = psg.tile([EC, BL * A], f32, tag="gath")
                    for h2 in range(2):
                        nc.tensor.matmul(
                            gps[:, h2 * 512:(h2 + 1) * 512],
                            gf_t[:, c * EC:(c + 1) * EC],
                            qsrc[:, h2 * 512:(h2 + 1) * 512],
                            start=True, stop=True)
                    cf_t = cfpool.tile([EC, BL, A], f32, tag=f"cf{c}",
                                       name=f"cf{c}_{k}")
                    nc.vector.tensor_tensor(
                        cf_t[:].rearrange("p b a -> p (b a)"), gps[:],
                        mb_t[c][:].rearrange("p b a -> p (b a)"), op=sub)

                    gps2 = psg.tile([EC, BL * A], f32, tag="gath")
                    for h2 in range(2):
                        nc.tensor.matmul(
                            gps2[:, h2 * 512:(h2 + 1) * 512],
                            gt_t[:, c * EC:(c + 1) * EC],
                            qsrc[:, h2 * 512:(h2 + 1) * 512],
                            start=True, stop=True)
                    cb_t = cfpool.tile([EC, BL, A], f32, tag=f"cb{c}",
                                       name=f"cb{c}_{k}")
                    nc.vector.tensor_tensor(
                        cb_t[:].rearrange("p b a -> p (b a)"), gps2[:],
                        mf_t[c][:].rearrange("p b a -> p (b a)"), op=sub)
                    cf_c.append(cf_t)
                    cb_c.append(cb_t)

                # Phase B: stream chunks, compute raw messages, mean-center.
                for c in range(NCHUNK):
                    cf_t = cf_c[c]
                    cb_t = cb_c[c]
                    mfraw = rpool.tile([EC, BL, A], f32, tag="mfraw")
                    mbraw = rpool.tile([EC, BL, A], f32, tag="mbraw")
                    for h in range(NBH):
                        et = epool.tile([EC, BH, A, A], f32, tag="et")
                        nc.sync.dma_start(
                            et[:],
                            edge_in[c * EC:(c + 1) * EC,
                                    h * BH:(h + 1) * BH, :, :])
                        bsl = slice(h * BH, (h + 1) * BH)
                        # forward: tmp = edge + cf[af] (bcast over at);
                        # max over af.  backward: tmp2 = edge + cb[at]
                        # (bcast over af); max over at.
                        # A subset of units runs on GPSIMD (tree-max) to
                        # offload the Vector engine; max order is exact.
                        fwd_gp = GP_UNITS.get((c, h, 0), False)
                        bwd_gp = GP_UNITS.get((c, h, 1), False)

                        if PE_FWD_ADD:
                            # fl(edge + cf) built on PE in PSUM pieces;
                            # DVE reduces straight from PSUM.
                            bpp = BH // PPC          # b per piece
                            et_flat = et[:].rearrange("p b af at -> p (b af at)")
                            for pc in range(PPC):
                                pp = pstmp.tile([EC, bpp * A * A], f32,
                                                tag="ptmp", name=f"ptmp{pc}")
                                for h2 in range(2):
                                    b2 = pc * bpp + h2 * (bpp // 2)
                                    sl = slice(h2 * 512, (h2 + 1) * 512)
                                    nc.tensor.matmul(
                                        pp[:, sl], ident_t[:],
                                        et_flat[:, b2 * 256:(b2 + bpp // 2)
                                                * 256],
                                        start=True, stop=False)
                                    nc.tensor.matmul(
                                        pp[:, sl], ident_t[:],
                                        cf_t[:, h * BH + b2:
                                             h * BH + b2 + bpp // 2, :, None]
                                        .to_broadcast(
                                            (EC, bpp // 2, A, A)),
                                        start=False, stop=True)
                                b0 = h * BH + pc * bpp
                                nc.vector.reduce_max(
                                    mfraw[:, b0:b0 + bpp, :],
                                    pp[:].rearrange(
                                        "p (b af at) -> p b at af",
                                        af=A, at=A),
                                    axis=mybir.AxisListType.X)
                        else:
                            eng = nc.gpsimd if fwd_gp else nc.vector
                            tmp = tpool.tile([EC, BH, A, A], f32, tag="tmp")
                            eng.tensor_tensor(
                                tmp[:], et[:],
                                cf_t[:, bsl, :, None].to_broadcast(
                                    (EC, BH, A, A)), op=add)
                            if fwd_gp:
                                _tree_max(nc.gpsimd, trpool, f32, tmp,
                                          mfraw[:, bsl, :], axis=2)
                            else:
                                nc.vector.reduce_max(
                                    mfraw[:, bsl, :],
                                    tmp[:].rearrange(
                                        "p b af at -> p b at af"),
                                    axis=mybir.AxisListType.X)

                        eng = nc.gpsimd if bwd_gp else nc.vector
                        tmp2 = tpool.tile([EC, BH, A, A], f32, tag="tmp")
                        eng.tensor_tensor(
                            tmp2[:], et[:],
                            cb_t[:, bsl, None, :].to_broadcast(
                                (EC, BH, A, A)), op=add)
                        if bwd_gp:
                            _tree_max(nc.gpsimd, trpool, f32, tmp2,
                                      mbraw[:, bsl, :], axis=3)
                        else:
                            nc.vector.reduce_max(
                                mbraw[:, bsl, :], tmp2[:],
                                axis=mybir.AxisListType.X)

                    # mean-center and store as the new messages
                    for raw, msg in ((mfraw, mf_t[c]), (mbraw, mb_t[c])):
                        msum = spool.tile([EC, BL], f32, tag="msum")
                        nc.vector.reduce_sum(msum[:], raw[:],
                                             axis=mybir.AxisListType.X)
                        mmean = spool.tile([EC, BL], f32, tag="mmean")
                        nc.vector.tensor_scalar_mul(mmean[:], msum[:],
                                                    1.0 / A)
                        nc.vector.tensor_tensor(
                            msg[:], raw[:],
                            mmean[:, :, None].to_broadcast((EC, BL, A)),
                            op=sub)

                    # scatter: q += S_to.T @ mf  +  S_from.T @ mb
                    for lhs, msg in ((st_c[c], mf_t[c]), (sf_c[c], mb_t[c])):
                        flat = msg[:].rearrange("p b a -> p (b a)")
                        for h2 in range(2):
                            nc.tensor.matmul(
                                qacc[:, h2 * 512:(h2 + 1) * 512],
                                lhs[:],
                                flat[:, h2 * 512:(h2 + 1) * 512],
                                start=(n_sc == 0),
                                stop=(n_sc == 2 * NCHUNK - 1))
                        n_sc += 1

                nc.vector.tensor_tensor(q_t[:], q0_t[:], qacc[:], op=add)
                nc.sync.dma_start(q_out[k, :, :], q_t[:])


def _get_program():
    if "nc" not in _prog_cache:
        _prog_cache["nc"] = _build_program()
    return _prog_cache["nc"]


def _make_in_maps(node_vals, edge_vals, ef, et):
    gf = np.zeros((N, E), np.float32)
    gf[ef, np.arange(E)] = 1.0
    gt = np.zeros((N, E), np.float32)
    gt[et, np.arange(E)] = 1.0
    sf = np.ascontiguousarray(gf.T)
    st = np.ascontiguousarray(gt.T)
    in_maps = []
    for c in range(NCORES):
        bs = slice(c * BL, (c + 1) * BL)
        edge_r = np.ascontiguousarray(edge_vals[bs].transpose(1, 0, 2, 3))
        q0 = np.ascontiguousarray(
            (node_vals[bs] * np.float32(E / N)).transpose(1, 0, 2)
        ).reshape(N, BL * A)
        in_maps.append({"edge": edge_r, "q0": q0, "gf": gf, "gt": gt,
                        "sf": sf, "st": st,
                        "ident": np.eye(EC, dtype=np.float32)})
    return in_maps


def _eval_actions(a, node_vals, edge_vals, ef, et):
    """Vectorized fp32 mirror of reference._eval_action. a: [B, N] int."""
    node_val = np.take_along_axis(
        node_vals, a[:, :, None], axis=-1)[..., 0].sum(-1, dtype=np.float32)
    a_f = a[:, ef]
    a_t = a[:, et]
    b_idx = np.arange(B)[:, None]
    e_idx = np.arange(E)[None, :]
    edge_val = edge_vals[b_idx, e_idx, a_f, a_t].sum(-1, dtype=np.float32)
    return (node_val / np.float32(N) + edge_val / np.float32(E)).astype(
        np.float32)


def _host_track(q_traj, node_vals, edge_vals, ef, et):
    """q_traj: [MSG_ITERS, B, N, A] scaled q values. Returns q_max, a_max."""
    a_max = np.argmax(node_vals, axis=-1).astype(np.int32)
    q_max = _eval_actions(a_max, node_vals, edge_vals, ef, et)
    for k in range(MSG_ITERS):
        a_k = np.argmax(q_traj[k], axis=-1).astype(np.int32)
        q_val = _eval_actions(a_k, node_vals, edge_vals, ef, et)
        upd = q_val > q_max
        a_max = np.where(upd[:, None], a_k, a_max)
        q_max = np.where(upd, q_val, q_max)
    return q_max.astype(np.float32), a_max.astype(np.int32)


def _run_device(in_maps, trace=False):
    from concourse.bass_utils import run_bass_kernel_spmd
    nc = _get_program()
    return run_bass_kernel_spmd(nc, in_maps, core_ids=list(range(NCORES)),
                                trace=trace)


def measure_hw_time(in_maps, n_iters=12):
    """Time repeated device executions (PJRT dispatch included), seconds."""
    import time
    import jax
    import numpy as _np
    from jax.sharding import Mesh, PartitionSpec
    from jax.experimental.shard_map import shard_map
    from concourse import bass2jax
    from concourse import mybir

    nc = _get_program()
    bass2jax.install_neuronx_cc_hook()
    partition_name = (nc.partition_id_tensor.name
                      if nc.partition_id_tensor else None)
    in_names, out_names, out_avals, zero_outs = [], [], [], []
    for alloc in nc.m.functions[0].allocations:
        if not isinstance(alloc, mybir.MemoryLocationSet):
            continue
        name = alloc.memorylocations[0].name
        if alloc.kind == "ExternalInput":
            if name != partition_name:
                in_names.append(name)
        elif alloc.kind == "ExternalOutput":
            shape = tuple(alloc.tensor_shape)
            dtype = mybir.dt.np(alloc.dtype)
            out_names.append(name)
            out_avals.append(jax.core.ShapedArray(shape, dtype))
            zero_outs.append(_np.zeros(shape, dtype))
    n_params = len(in_names)
    n_outs = len(out_avals)
    all_in_names = list(in_names) + list(out_names)
    if partition_name is not None:
        all_in_names.append(partition_name)

    def _body(*args):
        operands = list(args)
        if partition_name is not None:
            operands.append(bass2jax.partition_id_tensor())
        outs = bass2jax._bass_exec_p.bind(
            *operands,
            out_avals=tuple(out_avals),
            in_names=tuple(all_in_names),
            out_names=tuple(out_names),
            lowering_input_output_aliases=(),
            sim_require_finite=True,
            sim_require_nnan=True,
            nc=nc,
        )
        return tuple(outs)

    devices = jax.devices()[:NCORES]
    mesh = Mesh(_np.asarray(devices), ("core",))
    in_specs = (PartitionSpec("core"),) * (n_params + n_outs)
    out_specs = (PartitionSpec("core"),) * n_outs
    donate = tuple(range(n_params, n_params + n_outs))
    sharded = jax.jit(
        shard_map(_body, mesh=mesh, in_specs=in_specs, out_specs=out_specs,
                  check_rep=False),
        donate_argnums=donate, keep_unused=True)

    concat_in = [
        _np.concatenate([_np.asarray(m[name]) for m in in_maps], axis=0)
        for name in in_names
    ]
    sharding = jax.sharding.NamedSharding(mesh, PartitionSpec("core"))
    in_dev = [jax.device_put(a, sharding) for a in concat_in]

    times = []
    out = None
    for i in range(n_iters):
        zs = [jax.device_put(
            _np.zeros((NCORES * z.shape[0], *z.shape[1:]), z.dtype), sharding)
            for z in zero_outs]
        t0 = time.perf_counter()
        out = sharded(*in_dev, *zs)
        jax.block_until_ready(out)
        t1 = time.perf_counter()
        times.append(t1 - t0)
    return times


def kernel(node_vals, edge_vals, edges_from, edges_to):
    node_vals = np.asarray(node_vals, dtype=np.float32)
    edge_vals = np.asarray(edge_vals, dtype=np.float32)
    ef = np.asarray(edges_from, dtype=np.int64)
    et = np.asarray(edges_to, dtype=np.int64)

    in_maps = _make_in_maps(node_vals, edge_vals, ef, et)
    res = _run_device(in_maps)

    q_traj = np.empty((MSG_ITERS, B, N, A), np.float32)
    for c in range(NCORES):
        qo = res.results[c]["qout"].reshape(MSG_ITERS, N, BL, A)
        q_traj[:, c * BL:(c + 1) * BL] = qo.transpose(0, 2, 1, 3)

    q_max, a_max = _host_track(q_traj, node_vals, edge_vals, ef, et)
    return q_max, a_max
